# revision 30
# baseline (speedup 1.0000x reference)
"""Trainium2 Bass kernel for the BiLSTM-CRF loss (sum reduction).

Strategy (v5, host-transposed slot-major streaming, burn-free):
- Data-parallel: batch 256 sharded as 32 per NeuronCore across 8 cores.
- Normalizer runs in LINEAR space: alpha' = exp(em) .* (E^T alpha) with
  E = exp(transitions); 32 segments of 16 steps run concurrently as columns
  of one [128, 1024] chain. Interior segments are seeded directly from the
  predecessor's last-slot emission vector (one implicit power-iteration
  step; the strong contraction of E makes boundary error ~1e-5 relative).
- Emissions are host-packed SLOT-MAJOR and TRANSPOSED to [tag, (seg,b)]
  layout: each slot-pair DMAs straight into chain orientation; ACT exps a
  pair at a time (rescale 2^-8 folded into the exp bias) straight into the
  erm buffer. Chain round r consumes slot r and rides behind the stream.
- Per-round: 2 PE matmuls into one 2-bank PSUM tile + 1 DVE multiply.
- Segment growth telescopes via colsum tiles n (post-init) and m (final)
  kept in PSUM; tail is 5 ACT Ln+accum ops, WAW-chained onto one junk tile
  so the scheduler cannot hoist them into the Exp phase (table thrash).
- Numerator: transition/start/end scores via a host-built bigram count
  matrix; emission score via one indirect element gather (GPSIMD queue).

kernel() contract: full unsharded inputs in, full output (scalar) out.
"""
import numpy as np
import ml_dtypes

S, B, T = 512, 256, 128
NCORES, Bl = 8, 32
NSEG, SL = 32, 16
NR = SL                               # 16 rounds, no burn
LOG2C = -8.0
CBIAS = LOG2C * float(np.log(2.0))    # -5.5451774 (exp bias = log rescale)
CCORR = 32.0 * (31 * 16 + 15) * (-CBIAS)  # total rescale log correction
SLOT_ELEMS = NSEG * Bl * T            # 131072 elems per slot
EMFLAT_N = SL * SLOT_ELEMS + Bl * T   # slots + s=0 block
EM0_OFF = SL * SLOT_ELEMS
CHUNKS = [(15, 1), (0, 1), (1, 2), (3, 2), (5, 2), (7, 2), (9, 2),
          (11, 2), (13, 2)]          # (first slot, n slots) stream order

_NC = None


def _build():
    import concourse.bass as bass
    import concourse.tile as tile
    from concourse import bacc, mybir
    from contextlib import ExitStack

    f32 = mybir.dt.float32
    bf16 = mybir.dt.bfloat16
    i32 = mybir.dt.int32
    AF = mybir.ActivationFunctionType
    OP = mybir.AluOpType
    AX = mybir.AxisListType

    nc = bacc.Bacc("TRN2", target_bir_lowering=False, debug=False,
                   num_devices=NCORES)

    emflat = nc.dram_tensor("emflat", [EMFLAT_N, 1], bf16,
                            kind="ExternalInput")
    catv = nc.dram_tensor("catv", [T, 160], f32, kind="ExternalInput")
    catcnt = nc.dram_tensor("catcnt", [T, 130], f32, kind="ExternalInput")
    emidx = nc.dram_tensor("emidx", [128, 128], i32, kind="ExternalInput")
    outv = nc.dram_tensor("out", [1, 1], f32, kind="ExternalOutput")

    with tile.TileContext(nc) as tc, ExitStack() as ctx:
        const = ctx.enter_context(tc.tile_pool(name="const", bufs=1))
        stage = ctx.enter_context(tc.tile_pool(name="stage", bufs=1))
        pchain = ctx.enter_context(tc.tile_pool(name="pchain", bufs=1,
                                                space="PSUM"))
        pstat = ctx.enter_context(tc.tile_pool(name="pstat", bufs=1,
                                               space="PSUM"))

        # ---------- constants / small inputs (ACT HWDGE queue) ----------
        ones_col = const.tile([128, 1], bf16)
        nc.vector.memset(ones_col[:], 1.0)
        ones_colf = const.tile([128, 1], f32)
        nc.vector.memset(ones_colf[:], 1.0)
        cbias_col = const.tile([128, 1], f32)
        nc.vector.memset(cbias_col[:], CBIAS)

        catv_sb = const.tile([128, 160], f32)
        em0sb = const.tile([128, 32], bf16)
        catcnt_sb = const.tile([128, 130], f32)
        nc.gpsimd.dma_start(out=catcnt_sb[:], in_=catcnt[:, :])

        A = const.tile([128, NSEG, Bl], bf16)
        A2 = A.rearrange("p k b -> p (k b)")
        erm = const.tile([128, SL, NSEG, Bl], bf16)
        gem = const.tile([128, 128], bf16)

        E_hi = const.tile([128, 128], bf16)
        junkD = const.tile([1, 1], f32)
        nc.scalar.activation(junkD[:], ones_colf[0:1, :], AF.Exp)

        # ---------- slot-chunk DMA + per-slot exp (SBUF->SBUF) ----------
        def do_chunk(p):
            s0, ns = CHUNKS[p]
            natf = stage.tile([128, 2, 1024], bf16, tag="natf", bufs=3)
            nc.sync.dma_start(out=natf[:, 0:ns, :], in_=bass.AP(
                tensor=emflat, offset=s0 * SLOT_ELEMS,
                ap=[[1024, 128], [SLOT_ELEMS, ns], [1, 1024]]))
            for j in range(ns):
                nc.scalar.activation(erm[:, s0 + j, :, :], natf[:, j, :],
                                     AF.Exp, bias=cbias_col[:])

        # ---------- chain round: two pipelined half-chains ----------
        def do_half(r, h):
            last = r == NR - 1
            ka = 16 * h
            kb = min(16 * (h + 1), NSEG - 1 if last else NSEG)
            ps = pchain.tile([128, 512], f32, tag=f"ps{h}")
            w = (kb - ka) * Bl
            if r == 0 and h == 1:
                # round 0 upper half reads its seed vectors straight from erm
                rhs = erm[:, SL - 1, 15:31, :]
            else:
                rhs = A2[:, ka * Bl:kb * Bl]
            nc.tensor.matmul(out=ps[:, 0:w], lhsT=E_hi[:],
                             rhs=rhs,
                             start=True, stop=True)
            psv = ps.rearrange("p (k b) -> p k b", b=Bl)
            nc.vector.tensor_tensor(
                out=A[:, ka:kb, :], in0=psv[:, 0:kb - ka, :],
                in1=erm[:, r, ka:kb, :], op=OP.mult)

        def do_round(r):
            do_half(r, 0)
            do_half(r, 1)

        # ---------- emit: pairs, init, rounds pipelined ----------
        nps = None
        emitted = 0

        def emit_round():
            nonlocal emitted
            do_round(emitted)
            emitted += 1

        for p in range(len(CHUNKS)):
            do_chunk(p)
            if p == 0:
                # small inputs ride the sync queue right behind slot 15
                nc.sync.dma_start(out=catv_sb[:], in_=catv[:, :])
                nc.sync.dma_start(out=em0sb[:], in_=bass.AP(
                    tensor=emflat, offset=EM0_OFF, ap=[[32, 128], [1, 32]]))
                nc.scalar.activation(E_hi[:], catv_sb[:, 0:128], AF.Exp)
                # seed segment 0: exp(em0 + start) (ACT writes A col 0 first)
                nc.scalar.activation(A[:, 0, :], em0sb[:, 0:32], AF.Exp,
                                     bias=catv_sb[:, 128:129])
                # seed k=1..15 from erm[k-1, 15] (upper half reads erm
                # directly in round 0, so only the lower half needs A seeded)
                nc.vector.tensor_copy(out=A[:, 1:16, :],
                                      in_=erm[:, SL - 1, 0:15, :])
            elif p == 2:
                emit_round()
                emit_round()
                # n colsums for k>=1 straight from the erm seed vectors
                # (log n0 cancels in the telescoped logZ); these fill PE
                # idle slots between chain rounds
                nps = pstat.tile([1, 1024], f32, tag="nn")
                nc.tensor.matmul(out=nps[:, 32:512], lhsT=ones_col[:],
                                 rhs=erm[:, SL - 1, 0:15, :],
                                 start=True, stop=True)
                nc.tensor.matmul(out=nps[:, 512:1024], lhsT=ones_col[:],
                                 rhs=erm[:, SL - 1, 15:NSEG - 1, :],
                                 start=True, stop=True)
            elif p >= 4:
                emit_round()
                emit_round()
        Eend = const.tile([128, 1], bf16)
        nc.scalar.activation(Eend[:], catv_sb[:, 129:130], AF.Exp)

        # emission gather: emidx staged through the natf pool so the
        # indirect DMA cannot start until the stream is nearly done
        emidx_sb = stage.tile([128, 128], i32, tag="natf", bufs=3)
        nc.sync.dma_start(out=emidx_sb[:], in_=emidx[:, :])
        nc.gpsimd.indirect_dma_start(
            out=gem[:], out_offset=None,
            in_=bass.AP(tensor=emflat, offset=0, ap=[[1, EMFLAT_N], [1, 1]]),
            in_offset=bass.IndirectOffsetOnAxis(ap=emidx_sb[:], axis=0))

        # ---------- tail Ln ops, WAW-chained through one junk tile ------
        junkT = const.tile([1, 992], bf16)

        def ln_acc(name, src, width):
            acc = const.tile([1, 1], f32, name=f"a{name}")
            nc.scalar.activation(junkT[:, 0:width], src, AF.Ln,
                                 accum_out=acc[:])
            return acc

        # Ln table preload, pinned behind the final slot exp
        junk0 = const.tile([1, 1], f32)
        nc.scalar.activation(junkT[:, 0:1], erm[0:1, 14, 31, 31:32], AF.Ln,
                             accum_out=junk0[:])
        # n-side logs run during the trailing rounds (n0 cancels)
        nacc = ln_acc("n", nps[:, 32:992], 960)      # sum log n, 1<=k<=30
        n31acc = ln_acc("w", nps[:, 992:1024], 32)   # sum log n31

        while emitted < NR - 1:
            emit_round()
        # fin only needs segment 31, untouched by the last round
        finps = pstat.tile([1, 32], f32, tag="fx", bufs=2)
        nc.tensor.matmul(out=finps[:], lhsT=Eend[:],
                         rhs=A[:, NSEG - 1, :], start=True, stop=True)
        facc = ln_acc("f", finps[:], 32)             # sum log fin
        # n-side + fin combine, runs during the last round
        sn = const.tile([1, 1], f32)
        nc.vector.tensor_tensor(out=sn[:], in0=facc[:], in1=nacc[:],
                                op=OP.subtract)
        nc.vector.tensor_tensor(out=sn[:], in0=sn[:], in1=n31acc[:],
                                op=OP.subtract)
        nc.vector.tensor_scalar_add(sn[:], sn[:], CCORR)
        emit_round()

        # ---------- m stats ----------
        mps = pstat.tile([1, 1024], f32, tag="mm")
        for h in (0, 1):
            nc.tensor.matmul(out=mps[:, 512 * h:512 * (h + 1)],
                             lhsT=ones_col[:],
                             rhs=A2[:, 512 * h:512 * (h + 1)],
                             start=True, stop=True)

        # ---------- numerator reduce (off critical path) ----------
        gall = const.tile([1, 1], f32)
        nc.gpsimd.reduce_sum(out=gall[:], in_=gem[:], axis=AX.XYZWC)
        catp = const.tile([128, 130], f32)
        nc.gpsimd.tensor_tensor(out=catp[:], in0=catv_sb[:, 0:130],
                                in1=catcnt_sb[:], op=OP.mult)
        call = const.tile([1, 1], f32)
        nc.gpsimd.reduce_sum(out=call[:], in_=catp[:], axis=AX.XYZWC)
        numv = const.tile([1, 1], f32)
        nc.gpsimd.tensor_add(out=numv[:], in0=gall[:], in1=call[:])

        # ---------- final log + combine ----------
        gacc = ln_acc("g", mps[:, 0:992], 992)       # sum log m, k<=30
        s1 = const.tile([1, 1], f32)
        nc.vector.tensor_add(out=s1[:], in0=gacc[:], in1=sn[:])
        res = const.tile([1, 1], f32)
        nc.vector.tensor_tensor(out=res[:], in0=numv[:], in1=s1[:],
                                op=OP.subtract)
        nc.sync.dma_start(out=outv[:, :], in_=res[:])

    nc.compile()
    return nc


def _get_nc():
    global _NC
    if _NC is None:
        _NC = _build()
    return _NC


def make_in_maps(inputs):
    em = np.asarray(inputs["emissions"], dtype=np.float32)
    tags = np.asarray(inputs["tags"]).astype(np.int64)
    st = np.asarray(inputs["start_transitions"], dtype=np.float32)
    en = np.asarray(inputs["end_transitions"], dtype=np.float32)
    tr = np.asarray(inputs["transitions"], dtype=np.float32)
    catv = np.concatenate([tr, st[:, None], en[:, None],
                           np.zeros((T, 30), np.float32)],
                          axis=1).astype(np.float32)  # [T, 160] (64B rows)
    in_maps = []
    for c in range(NCORES):
        emc = em[:, c * Bl:(c + 1) * Bl, :]          # [S, Bl, T]
        tg = tags[:, c * Bl:(c + 1) * Bl]            # [S, Bl]
        # pack emflat[sl, t, k, b] = em[16k+sl+1, b, t]; tail: em0[t, b]
        pad = np.concatenate([emc[1:], np.zeros((1, Bl, T), np.float32)], 0)
        pk = pad.reshape(NSEG, SL, Bl, T)            # (k, sl, b, t)
        emh = np.ascontiguousarray(pk.transpose(1, 3, 0, 2))
        emflat = np.concatenate([emh.reshape(-1),
                                 np.ascontiguousarray(emc[0].T).reshape(-1)])
        emflat = emflat.astype(ml_dtypes.bfloat16)
        # emission gather indices (flat into emflat)
        s_all = np.arange(S)[:, None]
        b_all = np.arange(Bl)[None, :]
        k = (s_all - 1) // SL
        slx = (s_all - 1) % SL
        idx = slx * SLOT_ELEMS + tg * 1024 + k * 32 + b_all
        idx[0:1] = EM0_OFF + tg[0:1] * 32 + b_all
        # bigram + boundary counts
        cnt = np.zeros((T, 130), np.float32)
        np.add.at(cnt[:, 0:128], (tg[:-1].ravel(), tg[1:].ravel()), 1.0)
        np.add.at(cnt[:, 128], tg[0], 1.0)
        np.add.at(cnt[:, 129], tg[-1], 1.0)
        in_maps.append({
            "emflat": emflat.reshape(EMFLAT_N, 1),
            "catv": catv,
            "catcnt": cnt,
            "emidx": idx.astype(np.int32).reshape(128, 128),
        })
    return in_maps


def _numpy_fallback(inputs):
    """Exact float64 port of the reference (handles arbitrary masks)."""
    em = np.asarray(inputs["emissions"], dtype=np.float64)
    tags = np.asarray(inputs["tags"]).astype(np.int64)
    mask = np.asarray(inputs["mask"]).astype(bool)
    st = np.asarray(inputs["start_transitions"], dtype=np.float64)
    en = np.asarray(inputs["end_transitions"], dtype=np.float64)
    tr = np.asarray(inputs["transitions"], dtype=np.float64)
    Sl, Bn = tags.shape
    mask_f = mask.astype(np.float64)
    emit = np.take_along_axis(em, tags[:, :, None], axis=2)[:, :, 0]
    trsc = tr[tags[:-1], tags[1:]]
    score = st[tags[0]] + emit[0]
    score = score + ((trsc + emit[1:]) * mask_f[1:]).sum(0)
    seq_ends = mask.astype(np.int64).sum(0) - 1
    score = score + en[tags[seq_ends, np.arange(Bn)]]
    alpha = st[None, :] + em[0]
    for s in range(1, Sl):
        nxt = alpha[:, :, None] + tr[None] + em[s][:, None, :]
        mx = nxt.max(axis=1)
        nxt = mx + np.log(np.exp(nxt - mx[:, None, :]).sum(axis=1))
        alpha = np.where(mask[s][:, None], nxt, alpha)
    z = alpha + en[None, :]
    mz = z.max(axis=1)
    logZ = mz + np.log(np.exp(z - mz[:, None]).sum(axis=1))
    return np.asarray((score - logZ).sum(), dtype=np.float32)


def run_device(inputs, trace=False, trace_kwargs=None):
    from concourse.bass_utils import run_bass_kernel_spmd
    nc = _get_nc()
    in_maps = make_in_maps(inputs)
    try:
        br = run_bass_kernel_spmd(nc, in_maps, list(range(NCORES)),
                                  trace=trace, **(trace_kwargs or {}))
    except Exception:
        br = run_bass_kernel_spmd(nc, in_maps, list(range(NCORES)),
                                  trace=trace, **(trace_kwargs or {}))
    total = np.float32(
        sum(float(br.results[i]["out"][0, 0]) for i in range(NCORES)))
    return np.asarray(total, dtype=np.float32), br


def kernel(**inputs):
    mask = np.asarray(inputs["mask"])
    if not bool(mask.all()):
        return _numpy_fallback(inputs)
    val, _ = run_device(inputs, trace=False)
    return val


# revision 31
# speedup vs baseline: 1.0009x; 1.0009x over previous
"""Trainium2 Bass kernel for the BiLSTM-CRF loss (sum reduction).

Strategy (v5, host-transposed slot-major streaming, burn-free):
- Data-parallel: batch 256 sharded as 32 per NeuronCore across 8 cores.
- Normalizer runs in LINEAR space: alpha' = exp(em) .* (E^T alpha) with
  E = exp(transitions); 32 segments of 16 steps run concurrently as columns
  of one [128, 1024] chain. Interior segments are seeded directly from the
  predecessor's last-slot emission vector (one implicit power-iteration
  step; the strong contraction of E makes boundary error ~1e-5 relative).
- Emissions are host-packed SLOT-MAJOR and TRANSPOSED to [tag, (seg,b)]
  layout: each slot-pair DMAs straight into chain orientation; ACT exps a
  pair at a time (rescale 2^-8 folded into the exp bias) straight into the
  erm buffer. Chain round r consumes slot r and rides behind the stream.
- Per-round: 2 PE matmuls into one 2-bank PSUM tile + 1 DVE multiply.
- Segment growth telescopes via colsum tiles n (post-init) and m (final)
  kept in PSUM; tail is 5 ACT Ln+accum ops, WAW-chained onto one junk tile
  so the scheduler cannot hoist them into the Exp phase (table thrash).
- Numerator: transition/start/end scores via a host-built bigram count
  matrix; emission score via one indirect element gather (GPSIMD queue).

kernel() contract: full unsharded inputs in, full output (scalar) out.
"""
import numpy as np
import ml_dtypes

S, B, T = 512, 256, 128
NCORES, Bl = 8, 32
NSEG, SL = 32, 16
NR = SL                               # 16 rounds, no burn
LOG2C = -8.0
CBIAS = LOG2C * float(np.log(2.0))    # -5.5451774 (exp bias = log rescale)
CCORR = 32.0 * (31 * 16 + 15) * (-CBIAS)  # total rescale log correction
SLOT_ELEMS = NSEG * Bl * T            # 131072 elems per slot
EMFLAT_N = SL * SLOT_ELEMS + Bl * T   # slots + s=0 block
EM0_OFF = SL * SLOT_ELEMS
CHUNKS = [(15, 1), (0, 1), (1, 2), (3, 2), (5, 2), (7, 2), (9, 2),
          (11, 2), (13, 2)]          # (first slot, n slots) stream order

_NC = None


def _build():
    import concourse.bass as bass
    import concourse.tile as tile
    from concourse import bacc, mybir
    from contextlib import ExitStack

    f32 = mybir.dt.float32
    bf16 = mybir.dt.bfloat16
    i32 = mybir.dt.int32
    AF = mybir.ActivationFunctionType
    OP = mybir.AluOpType
    AX = mybir.AxisListType

    nc = bacc.Bacc("TRN2", target_bir_lowering=False, debug=False,
                   num_devices=NCORES)

    emflat = nc.dram_tensor("emflat", [EMFLAT_N, 1], bf16,
                            kind="ExternalInput")
    catv = nc.dram_tensor("catv", [T, 160], f32, kind="ExternalInput")
    catcnt = nc.dram_tensor("catcnt", [T, 130], f32, kind="ExternalInput")
    emidx = nc.dram_tensor("emidx", [128, 128], i32, kind="ExternalInput")
    outv = nc.dram_tensor("out", [1, 1], f32, kind="ExternalOutput")

    with tile.TileContext(nc) as tc, ExitStack() as ctx:
        const = ctx.enter_context(tc.tile_pool(name="const", bufs=1))
        stage = ctx.enter_context(tc.tile_pool(name="stage", bufs=1))
        pchain = ctx.enter_context(tc.tile_pool(name="pchain", bufs=1,
                                                space="PSUM"))
        pstat = ctx.enter_context(tc.tile_pool(name="pstat", bufs=1,
                                               space="PSUM"))

        # ---------- constants / small inputs (ACT HWDGE queue) ----------
        ones_col = const.tile([128, 1], bf16)
        nc.vector.memset(ones_col[:], 1.0)
        ones_colf = const.tile([128, 1], f32)
        nc.vector.memset(ones_colf[:], 1.0)
        cbias_col = const.tile([128, 1], f32)
        nc.vector.memset(cbias_col[:], CBIAS)

        catv_sb = const.tile([128, 160], f32)
        em0sb = const.tile([128, 32], bf16)
        catcnt_sb = const.tile([128, 130], f32)
        nc.gpsimd.dma_start(out=catcnt_sb[:], in_=catcnt[:, :])

        A = const.tile([128, NSEG, Bl], bf16)
        A2 = A.rearrange("p k b -> p (k b)")
        erm = const.tile([128, SL, NSEG, Bl], bf16)
        gem = const.tile([128, 128], bf16)

        E_hi = const.tile([128, 128], bf16)
        junkD = const.tile([1, 1], f32)
        nc.scalar.activation(junkD[:], ones_colf[0:1, :], AF.Exp)
        nc.scalar.activation(E_hi[:], catv_sb[:, 0:128], AF.Exp)

        # ---------- slot-chunk DMA + per-slot exp (SBUF->SBUF) ----------
        def do_chunk(p):
            s0, ns = CHUNKS[p]
            natf = stage.tile([128, 2, 1024], bf16, tag="natf", bufs=3)
            nc.sync.dma_start(out=natf[:, 0:ns, :], in_=bass.AP(
                tensor=emflat, offset=s0 * SLOT_ELEMS,
                ap=[[1024, 128], [SLOT_ELEMS, ns], [1, 1024]]))
            for j in range(ns):
                nc.scalar.activation(erm[:, s0 + j, :, :], natf[:, j, :],
                                     AF.Exp, bias=cbias_col[:])

        # ---------- chain round: two pipelined half-chains ----------
        def do_half(r, h):
            last = r == NR - 1
            ka = 16 * h
            kb = min(16 * (h + 1), NSEG - 1 if last else NSEG)
            ps = pchain.tile([128, 512], f32, tag=f"ps{h}")
            w = (kb - ka) * Bl
            if r == 0 and h == 1:
                # round 0 upper half reads its seed vectors straight from erm
                rhs = erm[:, SL - 1, 15:31, :]
            else:
                rhs = A2[:, ka * Bl:kb * Bl]
            nc.tensor.matmul(out=ps[:, 0:w], lhsT=E_hi[:],
                             rhs=rhs,
                             start=True, stop=True)
            psv = ps.rearrange("p (k b) -> p k b", b=Bl)
            nc.vector.tensor_tensor(
                out=A[:, ka:kb, :], in0=psv[:, 0:kb - ka, :],
                in1=erm[:, r, ka:kb, :], op=OP.mult)

        def do_round(r):
            do_half(r, 0)
            do_half(r, 1)

        # ---------- emit: pairs, init, rounds pipelined ----------
        nps = None
        emitted = 0

        def emit_round():
            nonlocal emitted
            do_round(emitted)
            emitted += 1

        for p in range(len(CHUNKS)):
            do_chunk(p)
            if p == 0:
                # small inputs ride the sync queue right behind slot 15
                nc.sync.dma_start(out=catv_sb[:], in_=catv[:, :])
                nc.sync.dma_start(out=em0sb[:], in_=bass.AP(
                    tensor=emflat, offset=EM0_OFF, ap=[[32, 128], [1, 32]]))
                # seed segment 0: exp(em0 + start) (ACT writes A col 0 first)
                nc.scalar.activation(A[:, 0, :], em0sb[:, 0:32], AF.Exp,
                                     bias=catv_sb[:, 128:129])
                # seed k=1..15 from erm[k-1, 15] (upper half reads erm
                # directly in round 0, so only the lower half needs A seeded)
                nc.vector.tensor_copy(out=A[:, 1:16, :],
                                      in_=erm[:, SL - 1, 0:15, :])
            elif p == 2:
                emit_round()
                emit_round()
                # n colsums for k>=1 straight from the erm seed vectors
                # (log n0 cancels in the telescoped logZ); these fill PE
                # idle slots between chain rounds
                nps = pstat.tile([1, 1024], f32, tag="nn")
                nc.tensor.matmul(out=nps[:, 32:512], lhsT=ones_col[:],
                                 rhs=erm[:, SL - 1, 0:15, :],
                                 start=True, stop=True)
                nc.tensor.matmul(out=nps[:, 512:1024], lhsT=ones_col[:],
                                 rhs=erm[:, SL - 1, 15:NSEG - 1, :],
                                 start=True, stop=True)
            elif p >= 4:
                emit_round()
                emit_round()
        Eend = const.tile([128, 1], bf16)
        nc.scalar.activation(Eend[:], catv_sb[:, 129:130], AF.Exp)

        # emission gather: emidx staged through the natf pool so the
        # indirect DMA cannot start until the stream is nearly done
        emidx_sb = stage.tile([128, 128], i32, tag="natf", bufs=3)
        nc.sync.dma_start(out=emidx_sb[:], in_=emidx[:, :])
        nc.gpsimd.indirect_dma_start(
            out=gem[:], out_offset=None,
            in_=bass.AP(tensor=emflat, offset=0, ap=[[1, EMFLAT_N], [1, 1]]),
            in_offset=bass.IndirectOffsetOnAxis(ap=emidx_sb[:], axis=0))

        # ---------- tail Ln ops, WAW-chained through one junk tile ------
        junkT = const.tile([1, 992], bf16)

        def ln_acc(name, src, width):
            acc = const.tile([1, 1], f32, name=f"a{name}")
            nc.scalar.activation(junkT[:, 0:width], src, AF.Ln,
                                 accum_out=acc[:])
            return acc

        # Ln table preload, pinned behind the final slot exp
        junk0 = const.tile([1, 1], f32)
        nc.scalar.activation(junkT[:, 0:1], erm[0:1, 14, 31, 31:32], AF.Ln,
                             accum_out=junk0[:])
        # n-side logs run during the trailing rounds (n0 cancels)
        nacc = ln_acc("n", nps[:, 32:992], 960)      # sum log n, 1<=k<=30
        n31acc = ln_acc("w", nps[:, 992:1024], 32)   # sum log n31

        while emitted < NR - 1:
            emit_round()
        # fin only needs segment 31, untouched by the last round
        finps = pstat.tile([1, 32], f32, tag="fx", bufs=2)
        nc.tensor.matmul(out=finps[:], lhsT=Eend[:],
                         rhs=A[:, NSEG - 1, :], start=True, stop=True)
        facc = ln_acc("f", finps[:], 32)             # sum log fin
        # n-side + fin combine, runs during the last round
        sn = const.tile([1, 1], f32)
        nc.vector.tensor_tensor(out=sn[:], in0=facc[:], in1=nacc[:],
                                op=OP.subtract)
        nc.vector.tensor_tensor(out=sn[:], in0=sn[:], in1=n31acc[:],
                                op=OP.subtract)
        nc.vector.tensor_scalar_add(sn[:], sn[:], CCORR)
        emit_round()

        # ---------- m stats ----------
        mps = pstat.tile([1, 1024], f32, tag="mm")
        for h in (0, 1):
            nc.tensor.matmul(out=mps[:, 512 * h:512 * (h + 1)],
                             lhsT=ones_col[:],
                             rhs=A2[:, 512 * h:512 * (h + 1)],
                             start=True, stop=True)

        # ---------- numerator reduce (off critical path) ----------
        gall = const.tile([1, 1], f32)
        nc.gpsimd.reduce_sum(out=gall[:], in_=gem[:], axis=AX.XYZWC)
        catp = const.tile([128, 130], f32)
        nc.gpsimd.tensor_tensor(out=catp[:], in0=catv_sb[:, 0:130],
                                in1=catcnt_sb[:], op=OP.mult)
        call = const.tile([1, 1], f32)
        nc.gpsimd.reduce_sum(out=call[:], in_=catp[:], axis=AX.XYZWC)
        numv = const.tile([1, 1], f32)
        nc.gpsimd.tensor_add(out=numv[:], in0=gall[:], in1=call[:])

        # ---------- final log + combine ----------
        gacc = ln_acc("g", mps[:, 0:992], 992)       # sum log m, k<=30
        s1 = const.tile([1, 1], f32)
        nc.vector.tensor_add(out=s1[:], in0=gacc[:], in1=sn[:])
        res = const.tile([1, 1], f32)
        nc.vector.tensor_tensor(out=res[:], in0=numv[:], in1=s1[:],
                                op=OP.subtract)
        nc.sync.dma_start(out=outv[:, :], in_=res[:])

    nc.compile()
    return nc


def _get_nc():
    global _NC
    if _NC is None:
        _NC = _build()
    return _NC


def make_in_maps(inputs):
    em = np.asarray(inputs["emissions"], dtype=np.float32)
    tags = np.asarray(inputs["tags"]).astype(np.int64)
    st = np.asarray(inputs["start_transitions"], dtype=np.float32)
    en = np.asarray(inputs["end_transitions"], dtype=np.float32)
    tr = np.asarray(inputs["transitions"], dtype=np.float32)
    catv = np.concatenate([tr, st[:, None], en[:, None],
                           np.zeros((T, 30), np.float32)],
                          axis=1).astype(np.float32)  # [T, 160] (64B rows)
    in_maps = []
    for c in range(NCORES):
        emc = em[:, c * Bl:(c + 1) * Bl, :]          # [S, Bl, T]
        tg = tags[:, c * Bl:(c + 1) * Bl]            # [S, Bl]
        # pack emflat[sl, t, k, b] = em[16k+sl+1, b, t]; tail: em0[t, b]
        pad = np.concatenate([emc[1:], np.zeros((1, Bl, T), np.float32)], 0)
        pk = pad.reshape(NSEG, SL, Bl, T)            # (k, sl, b, t)
        emh = np.ascontiguousarray(pk.transpose(1, 3, 0, 2))
        emflat = np.concatenate([emh.reshape(-1),
                                 np.ascontiguousarray(emc[0].T).reshape(-1)])
        emflat = emflat.astype(ml_dtypes.bfloat16)
        # emission gather indices (flat into emflat)
        s_all = np.arange(S)[:, None]
        b_all = np.arange(Bl)[None, :]
        k = (s_all - 1) // SL
        slx = (s_all - 1) % SL
        idx = slx * SLOT_ELEMS + tg * 1024 + k * 32 + b_all
        idx[0:1] = EM0_OFF + tg[0:1] * 32 + b_all
        # bigram + boundary counts
        cnt = np.zeros((T, 130), np.float32)
        np.add.at(cnt[:, 0:128], (tg[:-1].ravel(), tg[1:].ravel()), 1.0)
        np.add.at(cnt[:, 128], tg[0], 1.0)
        np.add.at(cnt[:, 129], tg[-1], 1.0)
        in_maps.append({
            "emflat": emflat.reshape(EMFLAT_N, 1),
            "catv": catv,
            "catcnt": cnt,
            "emidx": idx.astype(np.int32).reshape(128, 128),
        })
    return in_maps


def _numpy_fallback(inputs):
    """Exact float64 port of the reference (handles arbitrary masks)."""
    em = np.asarray(inputs["emissions"], dtype=np.float64)
    tags = np.asarray(inputs["tags"]).astype(np.int64)
    mask = np.asarray(inputs["mask"]).astype(bool)
    st = np.asarray(inputs["start_transitions"], dtype=np.float64)
    en = np.asarray(inputs["end_transitions"], dtype=np.float64)
    tr = np.asarray(inputs["transitions"], dtype=np.float64)
    Sl, Bn = tags.shape
    mask_f = mask.astype(np.float64)
    emit = np.take_along_axis(em, tags[:, :, None], axis=2)[:, :, 0]
    trsc = tr[tags[:-1], tags[1:]]
    score = st[tags[0]] + emit[0]
    score = score + ((trsc + emit[1:]) * mask_f[1:]).sum(0)
    seq_ends = mask.astype(np.int64).sum(0) - 1
    score = score + en[tags[seq_ends, np.arange(Bn)]]
    alpha = st[None, :] + em[0]
    for s in range(1, Sl):
        nxt = alpha[:, :, None] + tr[None] + em[s][:, None, :]
        mx = nxt.max(axis=1)
        nxt = mx + np.log(np.exp(nxt - mx[:, None, :]).sum(axis=1))
        alpha = np.where(mask[s][:, None], nxt, alpha)
    z = alpha + en[None, :]
    mz = z.max(axis=1)
    logZ = mz + np.log(np.exp(z - mz[:, None]).sum(axis=1))
    return np.asarray((score - logZ).sum(), dtype=np.float32)


def run_device(inputs, trace=False, trace_kwargs=None):
    from concourse.bass_utils import run_bass_kernel_spmd
    nc = _get_nc()
    in_maps = make_in_maps(inputs)
    try:
        br = run_bass_kernel_spmd(nc, in_maps, list(range(NCORES)),
                                  trace=trace, **(trace_kwargs or {}))
    except Exception:
        br = run_bass_kernel_spmd(nc, in_maps, list(range(NCORES)),
                                  trace=trace, **(trace_kwargs or {}))
    total = np.float32(
        sum(float(br.results[i]["out"][0, 0]) for i in range(NCORES)))
    return np.asarray(total, dtype=np.float32), br


def kernel(**inputs):
    mask = np.asarray(inputs["mask"])
    if not bool(mask.all()):
        return _numpy_fallback(inputs)
    val, _ = run_device(inputs, trace=False)
    return val


# revision 34
# speedup vs baseline: 1.0093x; 1.0084x over previous
"""Trainium2 Bass kernel for the BiLSTM-CRF loss (sum reduction).

Strategy (v5, host-transposed slot-major streaming, burn-free):
- Data-parallel: batch 256 sharded as 32 per NeuronCore across 8 cores.
- Normalizer runs in LINEAR space: alpha' = exp(em) .* (E^T alpha) with
  E = exp(transitions); 32 segments of 16 steps run concurrently as columns
  of one [128, 1024] chain. Interior segments are seeded directly from the
  predecessor's last-slot emission vector (one implicit power-iteration
  step; the strong contraction of E makes boundary error ~1e-5 relative).
- Emissions are host-packed SLOT-MAJOR and TRANSPOSED to [tag, (seg,b)]
  layout: each slot-pair DMAs straight into chain orientation; ACT exps a
  pair at a time (rescale 2^-8 folded into the exp bias) straight into the
  erm buffer. Chain round r consumes slot r and rides behind the stream.
- Per-round: 2 PE matmuls into one 2-bank PSUM tile + 1 DVE multiply.
- Segment growth telescopes via colsum tiles n (post-init) and m (final)
  kept in PSUM; tail is 5 ACT Ln+accum ops, WAW-chained onto one junk tile
  so the scheduler cannot hoist them into the Exp phase (table thrash).
- Numerator: transition/start/end scores via a host-built bigram count
  matrix; emission score via one indirect element gather (GPSIMD queue).

kernel() contract: full unsharded inputs in, full output (scalar) out.
"""
import numpy as np
import ml_dtypes

S, B, T = 512, 256, 128
NCORES, Bl = 8, 32
NSEG, SL = 32, 16
NR = SL                               # 16 rounds, no burn
LOG2C = -8.0
CBIAS = LOG2C * float(np.log(2.0))    # -5.5451774 (exp bias = log rescale)
CCORR = 32.0 * (31 * 16 + 15) * (-CBIAS)  # total rescale log correction
SLOT_ELEMS = NSEG * Bl * T            # 131072 elems per slot
EMFLAT_N = SL * SLOT_ELEMS + Bl * T   # slots + s=0 block
EM0_OFF = SL * SLOT_ELEMS
CHUNKS = [(15, 1), (0, 1), (1, 2), (3, 2), (5, 2), (7, 2), (9, 2),
          (11, 2), (13, 2)]          # (first slot, n slots) stream order

_NC = None


def _build():
    import concourse.bass as bass
    import concourse.tile as tile
    from concourse import bacc, mybir
    from contextlib import ExitStack

    f32 = mybir.dt.float32
    bf16 = mybir.dt.bfloat16
    i32 = mybir.dt.int32
    AF = mybir.ActivationFunctionType
    OP = mybir.AluOpType
    AX = mybir.AxisListType

    nc = bacc.Bacc("TRN2", target_bir_lowering=False, debug=False,
                   num_devices=NCORES)

    emflat = nc.dram_tensor("emflat", [EMFLAT_N, 1], bf16,
                            kind="ExternalInput")
    catv = nc.dram_tensor("catv", [T, 160], f32, kind="ExternalInput")
    catcnt = nc.dram_tensor("catcnt", [T, 130], f32, kind="ExternalInput")
    emidx = nc.dram_tensor("emidx", [128, 128], i32, kind="ExternalInput")
    outv = nc.dram_tensor("out", [1, 1], f32, kind="ExternalOutput")

    with tile.TileContext(nc) as tc, ExitStack() as ctx:
        const = ctx.enter_context(tc.tile_pool(name="const", bufs=1))
        stage = ctx.enter_context(tc.tile_pool(name="stage", bufs=1))
        pchain = ctx.enter_context(tc.tile_pool(name="pchain", bufs=1,
                                                space="PSUM"))
        pstat = ctx.enter_context(tc.tile_pool(name="pstat", bufs=1,
                                               space="PSUM"))

        # ---------- constants / small inputs (ACT HWDGE queue) ----------
        ones_col = const.tile([128, 1], bf16)
        nc.vector.memset(ones_col[:], 1.0)
        ones_colf = const.tile([128, 1], f32)
        nc.vector.memset(ones_colf[:], 1.0)
        cbias_col = const.tile([128, 1], f32)
        nc.vector.memset(cbias_col[:], CBIAS)

        catv_sb = const.tile([128, 160], f32)
        em0sb = const.tile([128, 32], bf16)
        catcnt_sb = const.tile([128, 130], f32)
        nc.gpsimd.dma_start(out=catcnt_sb[:], in_=catcnt[:, :])

        A = const.tile([128, NSEG, Bl], bf16)
        A2 = A.rearrange("p k b -> p (k b)")
        erm = const.tile([128, SL, NSEG, Bl], bf16)
        gem = const.tile([128, 128], bf16)

        E_hi = const.tile([128, 128], bf16)
        # slot 15 split column-wise across both HWDGE queues so it lands
        # ~1.5us sooner (dispatches emitted before any ACT compute)
        natf0 = stage.tile([128, 2, 1024], bf16, tag="natf", bufs=3)
        nc.scalar.dma_start(out=natf0[:, 0, 512:1024], in_=bass.AP(
            tensor=emflat, offset=15 * SLOT_ELEMS + 512,
            ap=[[1024, 128], [1, 512]]))
        nc.sync.dma_start(out=natf0[:, 0, 0:512], in_=bass.AP(
            tensor=emflat, offset=15 * SLOT_ELEMS,
            ap=[[1024, 128], [1, 512]]))
        natf1 = stage.tile([128, 2, 1024], bf16, tag="natf", bufs=3)
        nc.scalar.dma_start(out=natf1[:, 0, 512:1024], in_=bass.AP(
            tensor=emflat, offset=512, ap=[[1024, 128], [1, 512]]))
        nc.sync.dma_start(out=natf1[:, 0, 0:512], in_=bass.AP(
            tensor=emflat, offset=0, ap=[[1024, 128], [1, 512]]))
        junkD = const.tile([1, 1], f32)
        nc.scalar.activation(junkD[:], ones_colf[0:1, :], AF.Exp)

        # ---------- slot-chunk DMA + per-slot exp (SBUF->SBUF) ----------
        def do_chunk(p):
            s0, ns = CHUNKS[p]
            if p == 0:
                natf = natf0
            elif p == 1:
                natf = natf1
            else:
                natf = stage.tile([128, 2, 1024], bf16, tag="natf", bufs=3)
                nc.sync.dma_start(out=natf[:, 0:ns, :], in_=bass.AP(
                    tensor=emflat, offset=s0 * SLOT_ELEMS,
                    ap=[[1024, 128], [SLOT_ELEMS, ns], [1, 1024]]))
            for j in range(ns):
                nc.scalar.activation(erm[:, s0 + j, :, :], natf[:, j, :],
                                     AF.Exp, bias=cbias_col[:])

        # ---------- chain round: two pipelined half-chains ----------
        def do_half(r, h):
            last = r == NR - 1
            ka = 16 * h
            kb = min(16 * (h + 1), NSEG - 1 if last else NSEG)
            ps = pchain.tile([128, 512], f32, tag=f"ps{h}")
            w = (kb - ka) * Bl
            if r == 0 and h == 1:
                # round 0 upper half reads its seed vectors straight from erm
                rhs = erm[:, SL - 1, 15:31, :]
            else:
                rhs = A2[:, ka * Bl:kb * Bl]
            nc.tensor.matmul(out=ps[:, 0:w], lhsT=E_hi[:],
                             rhs=rhs,
                             start=True, stop=True)
            psv = ps.rearrange("p (k b) -> p k b", b=Bl)
            nc.vector.tensor_tensor(
                out=A[:, ka:kb, :], in0=psv[:, 0:kb - ka, :],
                in1=erm[:, r, ka:kb, :], op=OP.mult)

        def do_round(r):
            do_half(r, 0)
            do_half(r, 1)

        # ---------- emit: pairs, init, rounds pipelined ----------
        nps = None
        emitted = 0

        def emit_round():
            nonlocal emitted
            do_round(emitted)
            emitted += 1

        for p in range(len(CHUNKS)):
            do_chunk(p)
            if p == 0:
                # small inputs ride the sync queue right behind slot 15
                nc.sync.dma_start(out=catv_sb[:], in_=catv[:, :])
                nc.sync.dma_start(out=em0sb[:], in_=bass.AP(
                    tensor=emflat, offset=EM0_OFF, ap=[[32, 128], [1, 32]]))
            elif p == 1:
                # seed segment 0: exp(em0 + start) (ACT writes A col 0 first)
                nc.scalar.activation(A[:, 0, :], em0sb[:, 0:32], AF.Exp,
                                     bias=catv_sb[:, 128:129])
                nc.scalar.activation(E_hi[:], catv_sb[:, 0:128], AF.Exp)
                # seed k=1..15 from erm[k-1, 15] (upper half reads erm
                # directly in round 0, so only the lower half needs A seeded)
                nc.vector.tensor_copy(out=A[:, 1:16, :],
                                      in_=erm[:, SL - 1, 0:15, :])
            elif p == 2:
                emit_round()
                emit_round()
                # n colsums for k>=1 straight from the erm seed vectors
                # (log n0 cancels in the telescoped logZ); these fill PE
                # idle slots between chain rounds
                nps = pstat.tile([1, 1024], f32, tag="nn")
                nc.tensor.matmul(out=nps[:, 32:512], lhsT=ones_col[:],
                                 rhs=erm[:, SL - 1, 0:15, :],
                                 start=True, stop=True)
                nc.tensor.matmul(out=nps[:, 512:1024], lhsT=ones_col[:],
                                 rhs=erm[:, SL - 1, 15:NSEG - 1, :],
                                 start=True, stop=True)
            elif p >= 4:
                emit_round()
                emit_round()
        Eend = const.tile([128, 1], bf16)
        nc.scalar.activation(Eend[:], catv_sb[:, 129:130], AF.Exp)

        # emission gather: emidx staged through the natf pool so the
        # indirect DMA cannot start until the stream is nearly done
        emidx_sb = stage.tile([128, 128], i32, tag="natf", bufs=3)
        nc.sync.dma_start(out=emidx_sb[:], in_=emidx[:, :])
        nc.gpsimd.indirect_dma_start(
            out=gem[:], out_offset=None,
            in_=bass.AP(tensor=emflat, offset=0, ap=[[1, EMFLAT_N], [1, 1]]),
            in_offset=bass.IndirectOffsetOnAxis(ap=emidx_sb[:], axis=0))

        # ---------- tail Ln ops, WAW-chained through one junk tile ------
        junkT = const.tile([1, 992], bf16)

        def ln_acc(name, src, width):
            acc = const.tile([1, 1], f32, name=f"a{name}")
            nc.scalar.activation(junkT[:, 0:width], src, AF.Ln,
                                 accum_out=acc[:])
            return acc

        # Ln table preload, pinned behind the final slot exp
        junk0 = const.tile([1, 1], f32)
        nc.scalar.activation(junkT[:, 0:1], erm[0:1, 14, 31, 31:32], AF.Ln,
                             accum_out=junk0[:])
        # n-side logs run during the trailing rounds (n0 cancels)
        nacc = ln_acc("n", nps[:, 32:992], 960)      # sum log n, 1<=k<=30
        n31acc = ln_acc("w", nps[:, 992:1024], 32)   # sum log n31

        while emitted < NR - 1:
            emit_round()
        # fin only needs segment 31, untouched by the last round
        finps = pstat.tile([1, 32], f32, tag="fx", bufs=2)
        nc.tensor.matmul(out=finps[:], lhsT=Eend[:],
                         rhs=A[:, NSEG - 1, :], start=True, stop=True)
        facc = ln_acc("f", finps[:], 32)             # sum log fin
        # n-side + fin combine, runs during the last round
        sn = const.tile([1, 1], f32)
        nc.vector.tensor_tensor(out=sn[:], in0=facc[:], in1=nacc[:],
                                op=OP.subtract)
        nc.vector.tensor_tensor(out=sn[:], in0=sn[:], in1=n31acc[:],
                                op=OP.subtract)
        nc.vector.tensor_scalar_add(sn[:], sn[:], CCORR)
        emit_round()

        # ---------- m stats ----------
        mps = pstat.tile([1, 1024], f32, tag="mm")
        for h in (0, 1):
            nc.tensor.matmul(out=mps[:, 512 * h:512 * (h + 1)],
                             lhsT=ones_col[:],
                             rhs=A2[:, 512 * h:512 * (h + 1)],
                             start=True, stop=True)

        # ---------- numerator reduce (off critical path) ----------
        gall = const.tile([1, 1], f32)
        nc.gpsimd.reduce_sum(out=gall[:], in_=gem[:], axis=AX.XYZWC)
        catp = const.tile([128, 130], f32)
        nc.gpsimd.tensor_tensor(out=catp[:], in0=catv_sb[:, 0:130],
                                in1=catcnt_sb[:], op=OP.mult)
        call = const.tile([1, 1], f32)
        nc.gpsimd.reduce_sum(out=call[:], in_=catp[:], axis=AX.XYZWC)
        numv = const.tile([1, 1], f32)
        nc.gpsimd.tensor_add(out=numv[:], in0=gall[:], in1=call[:])

        # ---------- final log + combine ----------
        gacc = ln_acc("g", mps[:, 0:992], 992)       # sum log m, k<=30
        s1 = const.tile([1, 1], f32)
        nc.vector.tensor_add(out=s1[:], in0=gacc[:], in1=sn[:])
        res = const.tile([1, 1], f32)
        nc.vector.tensor_tensor(out=res[:], in0=numv[:], in1=s1[:],
                                op=OP.subtract)
        nc.sync.dma_start(out=outv[:, :], in_=res[:])

    nc.compile()
    return nc


def _get_nc():
    global _NC
    if _NC is None:
        _NC = _build()
    return _NC


def make_in_maps(inputs):
    em = np.asarray(inputs["emissions"], dtype=np.float32)
    tags = np.asarray(inputs["tags"]).astype(np.int64)
    st = np.asarray(inputs["start_transitions"], dtype=np.float32)
    en = np.asarray(inputs["end_transitions"], dtype=np.float32)
    tr = np.asarray(inputs["transitions"], dtype=np.float32)
    catv = np.concatenate([tr, st[:, None], en[:, None],
                           np.zeros((T, 30), np.float32)],
                          axis=1).astype(np.float32)  # [T, 160] (64B rows)
    in_maps = []
    for c in range(NCORES):
        emc = em[:, c * Bl:(c + 1) * Bl, :]          # [S, Bl, T]
        tg = tags[:, c * Bl:(c + 1) * Bl]            # [S, Bl]
        # pack emflat[sl, t, k, b] = em[16k+sl+1, b, t]; tail: em0[t, b]
        pad = np.concatenate([emc[1:], np.zeros((1, Bl, T), np.float32)], 0)
        pk = pad.reshape(NSEG, SL, Bl, T)            # (k, sl, b, t)
        emh = np.ascontiguousarray(pk.transpose(1, 3, 0, 2))
        emflat = np.concatenate([emh.reshape(-1),
                                 np.ascontiguousarray(emc[0].T).reshape(-1)])
        emflat = emflat.astype(ml_dtypes.bfloat16)
        # emission gather indices (flat into emflat)
        s_all = np.arange(S)[:, None]
        b_all = np.arange(Bl)[None, :]
        k = (s_all - 1) // SL
        slx = (s_all - 1) % SL
        idx = slx * SLOT_ELEMS + tg * 1024 + k * 32 + b_all
        idx[0:1] = EM0_OFF + tg[0:1] * 32 + b_all
        # bigram + boundary counts
        cnt = np.zeros((T, 130), np.float32)
        np.add.at(cnt[:, 0:128], (tg[:-1].ravel(), tg[1:].ravel()), 1.0)
        np.add.at(cnt[:, 128], tg[0], 1.0)
        np.add.at(cnt[:, 129], tg[-1], 1.0)
        in_maps.append({
            "emflat": emflat.reshape(EMFLAT_N, 1),
            "catv": catv,
            "catcnt": cnt,
            "emidx": idx.astype(np.int32).reshape(128, 128),
        })
    return in_maps


def _numpy_fallback(inputs):
    """Exact float64 port of the reference (handles arbitrary masks)."""
    em = np.asarray(inputs["emissions"], dtype=np.float64)
    tags = np.asarray(inputs["tags"]).astype(np.int64)
    mask = np.asarray(inputs["mask"]).astype(bool)
    st = np.asarray(inputs["start_transitions"], dtype=np.float64)
    en = np.asarray(inputs["end_transitions"], dtype=np.float64)
    tr = np.asarray(inputs["transitions"], dtype=np.float64)
    Sl, Bn = tags.shape
    mask_f = mask.astype(np.float64)
    emit = np.take_along_axis(em, tags[:, :, None], axis=2)[:, :, 0]
    trsc = tr[tags[:-1], tags[1:]]
    score = st[tags[0]] + emit[0]
    score = score + ((trsc + emit[1:]) * mask_f[1:]).sum(0)
    seq_ends = mask.astype(np.int64).sum(0) - 1
    score = score + en[tags[seq_ends, np.arange(Bn)]]
    alpha = st[None, :] + em[0]
    for s in range(1, Sl):
        nxt = alpha[:, :, None] + tr[None] + em[s][:, None, :]
        mx = nxt.max(axis=1)
        nxt = mx + np.log(np.exp(nxt - mx[:, None, :]).sum(axis=1))
        alpha = np.where(mask[s][:, None], nxt, alpha)
    z = alpha + en[None, :]
    mz = z.max(axis=1)
    logZ = mz + np.log(np.exp(z - mz[:, None]).sum(axis=1))
    return np.asarray((score - logZ).sum(), dtype=np.float32)


def run_device(inputs, trace=False, trace_kwargs=None):
    from concourse.bass_utils import run_bass_kernel_spmd
    nc = _get_nc()
    in_maps = make_in_maps(inputs)
    try:
        br = run_bass_kernel_spmd(nc, in_maps, list(range(NCORES)),
                                  trace=trace, **(trace_kwargs or {}))
    except Exception:
        br = run_bass_kernel_spmd(nc, in_maps, list(range(NCORES)),
                                  trace=trace, **(trace_kwargs or {}))
    total = np.float32(
        sum(float(br.results[i]["out"][0, 0]) for i in range(NCORES)))
    return np.asarray(total, dtype=np.float32), br


def kernel(**inputs):
    mask = np.asarray(inputs["mask"])
    if not bool(mask.all()):
        return _numpy_fallback(inputs)
    val, _ = run_device(inputs, trace=False)
    return val
